# revision 26
# baseline (speedup 1.0000x reference)
"""Multi-head attention kernel for Trainium2, sharded over 8 NeuronCores.

Problem: B=2, S=2048, D=1024, H=16 heads (DK=64).
  out = softmax(mask ? (XqWq^T)(XkWk^T)^T/8 : -1e9) (XvWv^T) Wo^T

Sharding: core c handles batch b=c//4 and 4 heads hg=c%4 (tensor parallel
over heads, data parallel over batch). Each core computes its partial
output projection y_t = Wo_slice^T-contribution [D, S]; the host sums the
4 partials per batch and transposes back.

On-chip layout is fully "transposed": projections are computed as
Qt = Wq_slice @ Xq^T -> [256, S] with head dim on partitions, so that
scores land as s_T[k, q] (keys on partitions) and the PV matmul consumes
the softmax probabilities directly as the moving operand -- no on-chip
transposes anywhere. Softmax denominators come for free from a ones
column appended to V (row 64 of the PV psum accumulates sum_k p[k,q]).

The scalar (ACT) engine is the roofline: 128 exp activations over
[128,1024] psum tiles ~= 147us that no other engine can run. The
schedule keeps ACT on exp only: all psum evictions run on DVE, and the
mask is applied differently per q-half to balance the other engines:
 - q-half 0: multiplicatively after exp (pt = exp * mask) on DVE/GPSIMD
 - q-half 1: additively before exp on the PE (an identity-stationary
   matmul accumulates maskneg = -1e9*(1-mask) into the scores psum;
   exp(0.125*(s-1e9)) == 0 exactly in fp32). This removes the DVE from
   the exp->PV chain and raises PE duty to ~94% so the HAM clock gate
   keeps the PE at 2.4GHz.
V-projection tiles are drip-fed through the scores-psum rotation during
(qh=0, hp=0) -- one [128,256] tile per kt step borrows the psum buffer
freed by that step's h2=0 exp -- so attention starts once K and half of
Q are projected instead of after all projections.
"""

import sys

sys.path.insert(0, "/opt/trn_rl_repo")

import numpy as np
import ml_dtypes
from contextlib import ExitStack

B, S, D, H = 2, 2048, 1024, 16
DK = D // H  # 64
N_CORES = 8
HPC = H // (N_CORES // B)  # 4 heads per core
EPC = HPC * DK  # 256 head-dims per core
P = 128
BF16 = ml_dtypes.bfloat16

_CACHE = {}


def _patch_tile_drain():
    """This walrus build allows only ONE sync-wait command on a Drain
    (CoreV3GenImpl setupSyncWait). Split the tail-drain waits across
    multiple drain instructions, one wait each."""
    import concourse.tile as tile
    from concourse import mybir
    from concourse.vector_clock import ScopedClock

    if getattr(tile.TileContext, "_drain_split_patch", False):
        return

    def _patched(self, tick_clock, wait_clock):
        nc = self.nc
        drain_inst = nc.sync.drain()
        wait_clock.add_sem_waits(
            drain_inst.ins, ScopedClock({None: tick_clock.global_clock})
        )
        si = drain_inst.ins.sync_info
        if si is not None and si.on_wait is not None and len(si.on_wait) > 1:
            extras = list(si.on_wait[1:])
            del si.on_wait[1:]
            for w in extras:
                d2 = nc.sync.drain()
                d2.ins.sync_info = mybir.SyncInfo(on_wait=[w], on_update=[])
        nc.all_engine_barrier()
        assert self.sems is not None
        popped = nc._tile_sem_poison_stack.pop()
        assert popped is self._sem_poison
        nc.clear_and_free_semaphores(list(self.sems.allocated().values()))
        nc.all_engine_barrier()

    tile.TileContext._drain_and_barrier = _patched
    tile.TileContext._drain_split_patch = True


def _split_multi_waits(nc):
    """This walrus build supports only ONE sync-wait command per
    instruction. Hoist extra waits onto preceding same-engine NoOps --
    engine sequencers process their stream in order, so a NoOp's wait
    gates everything after it on that engine."""
    from concourse import mybir

    ctr = [0]
    for fn in nc.m.functions:
        for blk in fn.blocks:
            insts = blk.instructions
            i = 0
            while i < len(insts):
                inst = insts[i]
                si = getattr(inst, "sync_info", None)
                waits = list(si.on_wait) if si is not None and si.on_wait else []
                if len(waits) > 1:
                    keep = waits[-1]
                    for w in waits[:-1]:
                        ctr[0] += 1
                        nop = mybir.InstNoOp(
                            name=f"wsplit_{ctr[0]}",
                            engine=inst.engine,
                            bass_nofuse=True,
                            sync_info=mybir.SyncInfo(on_wait=[w], on_update=[]),
                        )
                        insts.insert(i, nop)
                        i += 1
                    live = si.on_wait
                    del live[:-1]
                i += 1
    return ctr[0]


def _build_bass():
    import concourse.bass as bass
    import concourse.tile as tile
    from concourse import mybir

    _patch_tile_drain()
    bf16 = mybir.dt.bfloat16
    f32 = mybir.dt.float32
    Exp = mybir.ActivationFunctionType.Exp
    Copy = mybir.ActivationFunctionType.Copy

    nc = bass.Bass()
    xq = nc.dram_tensor("xq_t", [D, S], bf16, kind="ExternalInput")
    xk = nc.dram_tensor("xk_t", [D, S], bf16, kind="ExternalInput")
    xv = nc.dram_tensor("xv_t", [D, S], bf16, kind="ExternalInput")
    mk = nc.dram_tensor("mask_t", [S, S // 2], bf16, kind="ExternalInput")
    mn = nc.dram_tensor("maskneg_t", [S, S // 2], bf16, kind="ExternalInput")
    wq = nc.dram_tensor("wq_t", [D, EPC], bf16, kind="ExternalInput")
    wk = nc.dram_tensor("wk_t", [D, EPC], bf16, kind="ExternalInput")
    wv = nc.dram_tensor("wv_t", [D, EPC], bf16, kind="ExternalInput")
    wo = nc.dram_tensor("wo_t", [EPC, D], bf16, kind="ExternalInput")
    ident = nc.dram_tensor("ident_t", [P, P], bf16, kind="ExternalInput")
    y = nc.dram_tensor("y_t", [D, S], f32, kind="ExternalOutput")
    # DRAM scratch for partition-broadcasting softmax denominators
    # (SBUF->SBUF DMA cannot broadcast across partitions; DRAM can).
    rsum_dram = nc.dram_tensor("rsum_scratch", [8, 1024], f32, kind="Internal")
    rrec_dram = nc.dram_tensor("rrec_scratch", [8, 1024], f32, kind="Internal")

    KT = D // P  # 8 contraction tiles for projections
    ST = S // P  # 16 seq tiles
    VW = HPC * (DK + 1)  # 260: V columns + ones column per head

    with tile.TileContext(nc) as tc:
        with ExitStack() as ctx:
            # ---- pools (whole-kernel lifetime) ----
            # one x pool holds xk/xq/xv as full [128, 2048] tiles (4KB
            # DMA lines, ~2x the ring bandwidth of 2KB chunks). The mask
            # tiles are allocated from the SAME pool: they rotate into
            # the xk/xq buffers, which are dead after the projections.
            x_pool = ctx.enter_context(tc.tile_pool(name="x", bufs=24))
            w_pool = ctx.enter_context(tc.tile_pool(name="w", bufs=24))
            wo_pool = ctx.enter_context(tc.tile_pool(name="wo", bufs=2))
            id_pool = ctx.enter_context(tc.tile_pool(name="id", bufs=1))
            qt_pool = ctx.enter_context(tc.tile_pool(name="qt", bufs=2))
            kt_pool = ctx.enter_context(tc.tile_pool(name="kt", bufs=2))
            v_pool = ctx.enter_context(tc.tile_pool(name="v", bufs=ST))
            out_pool = ctx.enter_context(tc.tile_pool(name="outsb", bufs=2))
            exp_pool = ctx.enter_context(tc.tile_pool(name="exp", bufs=6))
            p_pool = ctx.enter_context(tc.tile_pool(name="p", bufs=8))
            otmp_pool = ctx.enter_context(tc.tile_pool(name="otmp", bufs=2))
            bc_pool = ctx.enter_context(tc.tile_pool(name="bc", bufs=2))
            r_pool = ctx.enter_context(tc.tile_pool(name="r", bufs=4))
            yev_pool = ctx.enter_context(tc.tile_pool(name="yev", bufs=4))
            dmy_pool = ctx.enter_context(tc.tile_pool(name="dmy", bufs=1))
            ps_s = ctx.enter_context(tc.tile_pool(name="ps_s", bufs=2, space="PSUM"))
            ps_o = ctx.enter_context(tc.tile_pool(name="ps_o", bufs=2, space="PSUM"))

            # ---- dummy exp: pull the ACT table load off the critical path
            dmy = dmy_pool.tile([P, 8], bf16, tag="dmy", name="dmy")
            nc.vector.memset(dmy[:], 0.0)
            nc.scalar.activation(dmy[:], dmy[:], Exp)

            # ---- DMAs across THREE queues. Pre-first-exp critical set
            # (wk/wq, xk, xq) splits across sync+scalar; xv/wv/wo/id go
            # on the GPSIMD SWDGE queue so the scalar queue drains before
            # the first exp (exps are in-order behind any scalar DMA).
            w_sb = {}

            def load_w(wname, t, q):
                tiles = []
                for k in range(KT):
                    wt = w_pool.tile([P, EPC], bf16, tag="w", name="w")
                    q.dma_start(wt[:], t[k * P:(k + 1) * P, :])
                    tiles.append(wt)
                w_sb[wname] = tiles

            def load_x_full(src, k, q):
                t = x_pool.tile([P, S], bf16, tag="x", name="x")
                q.dma_start(t[:], src[k * P:(k + 1) * P, :])
                return t

            load_w("wk", wk, nc.sync)
            load_w("wq", wq, nc.scalar)
            xk_sb = [load_x_full(xk, k, nc.sync if k % 2 == 0 else nc.scalar)
                     for k in range(KT)]
            xq_sb = [load_x_full(xq, k, nc.sync if k % 2 == 0 else nc.scalar)
                     for k in range(KT)]
            load_w("wv", wv, nc.gpsimd)
            xv_sb = [load_x_full(xv, k, nc.gpsimd) for k in range(KT)]
            wo_sb = []
            for k in range(2):
                wt = wo_pool.tile([P, D], bf16, tag="wo", name="wo")
                nc.gpsimd.dma_start(wt[:], wo[k * P:(k + 1) * P, :])
                wo_sb.append(wt)
            id_sb = id_pool.tile([P, P], bf16, tag="id", name="id")
            nc.gpsimd.dma_start(id_sb[:], ident[:, :])

            # ---- projection helpers ----
            qt_sb = [qt_pool.tile([P, S], bf16, tag="qt", name="qt") for _ in range(2)]
            kt_sb = [kt_pool.tile([P, S], bf16, tag="kt", name="kt") for _ in range(2)]

            def emit_qk_proj(dst_tiles, wname, x_sb, q4, m):
                """One (m, q4) psum group for the Q or K projection."""
                sl = slice(q4 * 512, (q4 + 1) * 512)
                ps = ps_s.tile([P, 1024], f32, tag="ps_s", name="ps_s")
                for k in range(KT):
                    nc.tensor.matmul(
                        ps[:, 0:512],
                        lhsT=w_sb[wname][k][:, m * P:(m + 1) * P],
                        rhs=x_sb[k][:, sl],
                        start=(k == 0),
                        stop=(k == KT - 1),
                    )
                nc.vector.tensor_copy(dst_tiles[m][:, sl], ps[:, 0:512])

            v_sb = [None] * ST

            def emit_v_proj(m):
                """V tile m: [128 k, 260] with a ones column per head."""
                vt = v_pool.tile([P, VW], bf16, tag="v", name="v")
                ps = ps_s.tile([P, 1024], f32, tag="ps_s", name="ps_s")
                for k in range(KT):
                    nc.tensor.matmul(
                        ps[:, 0:EPC],
                        lhsT=xv_sb[k][:, m * P:(m + 1) * P],
                        rhs=w_sb["wv"][k][:, :],
                        start=(k == 0),
                        stop=(k == KT - 1),
                    )
                vs = vt[:].rearrange("p (h x) -> p h x", h=HPC)
                # eviction on ACT: B\'s DVE is loaded with the mask muls
                # and the psum buffer must free at a deterministic ACT
                # slot right after the exps to keep the rotation paced
                nc.scalar.activation(
                    vs[:, :, 0:DK],
                    ps[:, 0:EPC].rearrange("p (h x) -> p h x", h=HPC),
                    Copy,
                )
                nc.vector.memset(vs[:, :, DK:DK + 1], 1.0)
                v_sb[m] = vt

            # ---- lead-in: K proj and FULL Q proj (both q-halves) ----
            for q4 in range(4):
                for m in range(2):
                    emit_qk_proj(kt_sb, "wk", xk_sb, q4, m)
            for q4 in range(4):
                for m in range(2):
                    emit_qk_proj(qt_sb, "wq", xq_sb, q4, m)

            # masks for qh=0 on sync, reusing dead xk/xq buffers
            mask_sb = []
            for kt in range(ST):
                mt = x_pool.tile([P, 1024], bf16, tag="x", name="mask")
                nc.sync.dma_start(mt[:], mk[kt * P:(kt + 1) * P, :])
                mask_sb.append(mt)

            # ---- output projection ----
            out_sb = [out_pool.tile([P, S], bf16, tag="outsb", name="outsb")
                      for _ in range(2)]

            def emit_oproj_chunk(m, qc, q0, evict_act, pool=None):
                sl = slice(q0 + qc * 512, q0 + (qc + 1) * 512)
                if pool is None:
                    pool = ps_s if qc == 0 else ps_o
                ps = pool.tile([P, 512], f32,
                               tag="ps_s" if pool is ps_s else "ps_o",
                               name="ps_op")
                for k in range(2):
                    nc.tensor.matmul(
                        ps[:, 0:512],
                        lhsT=wo_sb[k][:, m * P:(m + 1) * P],
                        rhs=out_sb[k][:, sl],
                        start=(k == 0),
                        stop=(k == 1),
                    )
                ev = yev_pool.tile([P, 512], f32, tag="yev", name="yev")
                if evict_act:
                    nc.scalar.activation(ev[:], ps[:, 0:512], Copy)
                    nc.sync.dma_start(y[m * P:(m + 1) * P, sl], ev[:])
                else:
                    nc.vector.tensor_copy(ev[:], ps[:, 0:512])
                    nc.scalar.dma_start(y[m * P:(m + 1) * P, sl], ev[:])

            # ---- attention ----
            maskneg_sb = [None] * ST

            def attention(qh, hp, inject):
                """One (q-half, head-pair-group) pass. inject: one
                optional callable per kt step, emitted after that step\'s
                PV so the PE never head-of-line blocks on it."""
                po = [ps_o.tile([DK + 1, 1024], f32, tag="ps_o", name="ps_o")
                      for _ in range(2)]

                def emit_pv(kt, pts):
                    for h2 in range(2):
                        h = 2 * hp + h2
                        for qc in range(2):
                            nc.tensor.matmul(
                                po[h2][:, qc * 512:(qc + 1) * 512],
                                lhsT=v_sb[kt][:, h * (DK + 1):(h + 1) * (DK + 1)],
                                rhs=pts[h2][:, qc * 512:(qc + 1) * 512],
                                start=(kt == 0),
                                stop=(kt == 15),
                            )

                q0 = qh * 1024
                prev = None
                for kt in range(ST):
                    psts = [ps_s.tile([P, 1024], f32, tag="ps_s", name="ps_s")
                            for _ in range(2)]
                    if qh == 1:
                        # additive mask: identity-stationary matmuls seed
                        # the psum with -1e9 on masked elements
                        for h2 in range(2):
                            for qc in range(2):
                                nc.tensor.matmul(
                                    psts[h2][:, qc * 512:(qc + 1) * 512],
                                    lhsT=id_sb[:],
                                    rhs=maskneg_sb[kt][:, qc * 512:(qc + 1) * 512],
                                    start=True,
                                    stop=False,
                                )
                    # scores: qc-outer / h2-inner so the two half-array
                    # matmuls (rows 0-63 vs 64-127) run concurrently
                    for qc in range(2):
                        for h2 in range(2):
                            r0 = h2 * DK
                            nc.tensor.matmul(
                                psts[h2][:, qc * 512:(qc + 1) * 512],
                                lhsT=kt_sb[hp][r0:r0 + DK, kt * P:(kt + 1) * P],
                                rhs=qt_sb[hp][r0:r0 + DK,
                                              q0 + qc * 512:q0 + (qc + 1) * 512],
                                start=(qh == 0),
                                stop=True,
                                tile_position=(r0, 0),
                            )
                    pts = []
                    for h2 in range(2):
                        et = exp_pool.tile([P, 1024], bf16, tag="exp", name="exp")
                        nc.scalar.activation(et[:], psts[h2][:], Exp, scale=0.125)
                        if qh == 0:
                            pt = p_pool.tile([P, 1024], bf16, tag="p", name="p")
                            nc.vector.tensor_mul(pt[:], et[:], mask_sb[kt][:])
                            pts.append(pt)
                        else:
                            pts.append(et)
                    # prefetch next-half (additive) mask during (qh0, hp1)
                    if qh == 0 and hp == 1:
                        mt = x_pool.tile([P, 1024], bf16, tag="x", name="maskn")
                        nc.sync.dma_start(mt[:], mn[kt * P:(kt + 1) * P, :])
                        maskneg_sb[kt] = mt
                    if prev is not None:
                        emit_pv(kt - 1, prev)
                    if inject[kt] is not None:
                        inject[kt]()
                    prev = pts
                emit_pv(15, prev)

                # normalize: row DK of po is sum_k p[k, q]. The chained
                # DMAs run on the GPSIMD (SWDGE) queue: their round-trip
                # semaphore waits would head-of-line block every later
                # transfer on a HWDGE queue. The two h2 chains are
                # interleaved stage-by-stage so the round trips overlap.
                ots, rqs, bcs, rrs = [], [], [], []
                for h2 in range(2):
                    ot = otmp_pool.tile([DK + 1, 1024], f32, tag="otmp",
                                        name="otmp")
                    nc.vector.tensor_copy(ot[:], po[h2][:])
                    ots.append(ot)
                for h2 in range(2):
                    ridx = (qh * 2 + hp) * 2 + h2
                    nc.gpsimd.dma_start(
                        rsum_dram[ridx:ridx + 1, :], ots[h2][DK:DK + 1, :])
                for h2 in range(2):
                    ridx = (qh * 2 + hp) * 2 + h2
                    rr = r_pool.tile([P, 8], f32, tag="r", name="rr")
                    nc.gpsimd.dma_start(
                        rr[:],
                        rsum_dram[ridx:ridx + 1, :].rearrange(
                            "o (p f) -> (o p) f", p=P),
                    )
                    rrs.append(rr)
                for h2 in range(2):
                    rq = r_pool.tile([P, 8], f32, tag="r", name="rq")
                    nc.vector.reciprocal(rq[:], rrs[h2][:])
                    rqs.append(rq)
                for h2 in range(2):
                    ridx = (qh * 2 + hp) * 2 + h2
                    nc.gpsimd.dma_start(
                        rrec_dram[ridx:ridx + 1, :].rearrange(
                            "o (p f) -> (o p) f", p=P),
                        rqs[h2][:],
                    )
                for h2 in range(2):
                    ridx = (qh * 2 + hp) * 2 + h2
                    bc = bc_pool.tile([DK, 1024], f32, tag="bc", name="bc")
                    nc.gpsimd.dma_start(
                        bc[:],
                        rrec_dram[ridx:ridx + 1, :].broadcast_to([DK, 1024]),
                    )
                    bcs.append(bc)
                for h2 in range(2):
                    nc.gpsimd.tensor_mul(
                        out_sb[hp][h2 * DK:(h2 + 1) * DK, q0:q0 + 1024],
                        ots[h2][0:DK, :],
                        bcs[h2][:],
                    )

            # phase B: (qh=0, hp=0), V projection injected 1 tile per kt
            attention(0, 0, [lambda m=m: emit_v_proj(m) for m in range(ST)])
            # phase C: (qh=0, hp=1) -- clean, Q was fully projected in lead
            attention(0, 1, [None] * ST)
            # phases E, F: (qh=1, hp=0/1), additive mask
            attention(1, 0, [None] * ST)
            attention(1, 1, [None] * ST)
            # tail: all 64 output-projection chunks, dense. q-half 0
            # first (its normalization finished two phases ago) so the
            # PE streams while F\'s normalization chain completes.
            i = 0
            for q0 in (0, 1024):
                for m in range(D // P):
                    for qc in range(2):
                        emit_oproj_chunk(m, qc, q0,
                                         evict_act=(i % 2 == 0))
                        i += 1

    _split_multi_waits(nc)
    return nc


def _get_nc():
    if "nc" not in _CACHE:
        _CACHE["nc"] = _build_bass()
    return _CACHE["nc"]


def kernel(query, key, value, mask, w_q, w_k, w_v, w_o, **unused):
    nc = _get_nc()
    from concourse.bass_utils import run_bass_kernel_spmd

    ident = np.eye(P, dtype=BF16)
    in_maps = []
    for c in range(N_CORES):
        b = c // (N_CORES // B)
        hg = c % (N_CORES // B)
        e0 = hg * EPC
        mt = np.ascontiguousarray(mask[b].T).astype(np.float32)
        in_maps.append({
            "xq_t": np.ascontiguousarray(query[b].T).astype(BF16),
            "xk_t": np.ascontiguousarray(key[b].T).astype(BF16),
            "xv_t": np.ascontiguousarray(value[b].T).astype(BF16),
            "mask_t": np.ascontiguousarray(mt[:, 0:1024]).astype(BF16),
            "maskneg_t": np.ascontiguousarray(
                (mt[:, 1024:2048] - 1.0) * 1e9).astype(BF16),
            "ident_t": ident,
            "wq_t": np.ascontiguousarray(w_q[e0:e0 + EPC, :].T).astype(BF16),
            "wk_t": np.ascontiguousarray(w_k[e0:e0 + EPC, :].T).astype(BF16),
            "wv_t": np.ascontiguousarray(w_v[e0:e0 + EPC, :].T).astype(BF16),
            "wo_t": np.ascontiguousarray(w_o[:, e0:e0 + EPC].T).astype(BF16),
        })

    res = run_bass_kernel_spmd(nc, in_maps, core_ids=list(range(N_CORES)))
    _CACHE["last_results"] = res

    gpb = N_CORES // B
    out = np.empty((B, S, D), dtype=np.float32)
    for b in range(B):
        acc = res.results[b * gpb]["y_t"].astype(np.float32)
        for c in range(b * gpb + 1, (b + 1) * gpb):
            acc = acc + res.results[c]["y_t"]
        out[b] = acc.T
    return out


# revision 27
# speedup vs baseline: 1.2802x; 1.2802x over previous
"""Multi-head attention kernel for Trainium2, sharded over 8 NeuronCores.

Problem: B=2, S=2048, D=1024, H=16 heads (DK=64).
  out = softmax(mask ? (XqWq^T)(XkWk^T)^T/8 : -1e9) (XvWv^T) Wo^T

Sharding: core c handles batch b=c//4 and 4 heads hg=c%4 (tensor parallel
over heads, data parallel over batch). Each core computes its partial
output projection y_t = Wo_slice^T-contribution [D, S]; the host sums the
4 partials per batch and transposes back.

On-chip layout is fully "transposed": projections are computed as
Qt = Wq_slice @ Xq^T -> [256, S] with head dim on partitions, so that
scores land as s_T[k, q] (keys on partitions) and the PV matmul consumes
the softmax probabilities directly as the moving operand -- no on-chip
transposes anywhere. Softmax denominators come for free from a ones
column appended to V (row 64 of the PV psum accumulates sum_k p[k,q]).

The scalar (ACT) engine is the roofline: 128 exp activations over
[128,1024] psum tiles ~= 147us that no other engine can run. The
schedule keeps ACT on exp only: all psum evictions run on DVE, and the
mask is applied differently per q-half to balance the other engines:
 - q-half 0: multiplicatively after exp (pt = exp * mask) on DVE/GPSIMD
 - q-half 1: additively before exp on the PE (an identity-stationary
   matmul accumulates maskneg = -1e9*(1-mask) into the scores psum;
   exp(0.125*(s-1e9)) == 0 exactly in fp32). This removes the DVE from
   the exp->PV chain and raises PE duty to ~94% so the HAM clock gate
   keeps the PE at 2.4GHz.
V-projection tiles are drip-fed through the scores-psum rotation during
(qh=0, hp=0) -- one [128,256] tile per kt step borrows the psum buffer
freed by that step's h2=0 exp -- so attention starts once K and half of
Q are projected instead of after all projections.
"""

import sys

sys.path.insert(0, "/opt/trn_rl_repo")

import numpy as np
import ml_dtypes
from contextlib import ExitStack

B, S, D, H = 2, 2048, 1024, 16
DK = D // H  # 64
N_CORES = 8
HPC = H // (N_CORES // B)  # 4 heads per core
EPC = HPC * DK  # 256 head-dims per core
P = 128
BF16 = ml_dtypes.bfloat16

_CACHE = {}


def _patch_tile_drain():
    """This walrus build allows only ONE sync-wait command on a Drain
    (CoreV3GenImpl setupSyncWait). Split the tail-drain waits across
    multiple drain instructions, one wait each."""
    import concourse.tile as tile
    from concourse import mybir
    from concourse.vector_clock import ScopedClock

    if getattr(tile.TileContext, "_drain_split_patch", False):
        return

    def _patched(self, tick_clock, wait_clock):
        nc = self.nc
        drain_inst = nc.sync.drain()
        wait_clock.add_sem_waits(
            drain_inst.ins, ScopedClock({None: tick_clock.global_clock})
        )
        si = drain_inst.ins.sync_info
        if si is not None and si.on_wait is not None and len(si.on_wait) > 1:
            extras = list(si.on_wait[1:])
            del si.on_wait[1:]
            for w in extras:
                d2 = nc.sync.drain()
                d2.ins.sync_info = mybir.SyncInfo(on_wait=[w], on_update=[])
        nc.all_engine_barrier()
        assert self.sems is not None
        popped = nc._tile_sem_poison_stack.pop()
        assert popped is self._sem_poison
        nc.clear_and_free_semaphores(list(self.sems.allocated().values()))
        nc.all_engine_barrier()

    tile.TileContext._drain_and_barrier = _patched
    tile.TileContext._drain_split_patch = True


def _split_multi_waits(nc):
    """This walrus build supports only ONE sync-wait command per
    instruction. Hoist extra waits onto preceding same-engine NoOps --
    engine sequencers process their stream in order, so a NoOp's wait
    gates everything after it on that engine."""
    from concourse import mybir

    ctr = [0]
    for fn in nc.m.functions:
        for blk in fn.blocks:
            insts = blk.instructions
            i = 0
            while i < len(insts):
                inst = insts[i]
                si = getattr(inst, "sync_info", None)
                waits = list(si.on_wait) if si is not None and si.on_wait else []
                if len(waits) > 1:
                    keep = waits[-1]
                    for w in waits[:-1]:
                        ctr[0] += 1
                        nop = mybir.InstNoOp(
                            name=f"wsplit_{ctr[0]}",
                            engine=inst.engine,
                            bass_nofuse=True,
                            sync_info=mybir.SyncInfo(on_wait=[w], on_update=[]),
                        )
                        insts.insert(i, nop)
                        i += 1
                    live = si.on_wait
                    del live[:-1]
                i += 1
    return ctr[0]


def _build_bass():
    import concourse.bass as bass
    import concourse.tile as tile
    from concourse import mybir

    _patch_tile_drain()
    bf16 = mybir.dt.bfloat16
    f32 = mybir.dt.float32
    Exp = mybir.ActivationFunctionType.Exp
    Copy = mybir.ActivationFunctionType.Copy

    nc = bass.Bass()
    xq = nc.dram_tensor("xq_t", [D, S], bf16, kind="ExternalInput")
    xk = nc.dram_tensor("xk_t", [D, S], bf16, kind="ExternalInput")
    xv = nc.dram_tensor("xv_t", [D, S], bf16, kind="ExternalInput")
    mk = nc.dram_tensor("mask_t", [S, S // 2], bf16, kind="ExternalInput")
    mn = nc.dram_tensor("maskneg_t", [S, S // 2], bf16, kind="ExternalInput")
    wq = nc.dram_tensor("wq_t", [D, EPC], bf16, kind="ExternalInput")
    wk = nc.dram_tensor("wk_t", [D, EPC], bf16, kind="ExternalInput")
    wv = nc.dram_tensor("wv_t", [D, EPC], bf16, kind="ExternalInput")
    wo = nc.dram_tensor("wo_t", [EPC, D], bf16, kind="ExternalInput")
    ident = nc.dram_tensor("ident_t", [P, P], bf16, kind="ExternalInput")
    y = nc.dram_tensor("y_t", [D, S], f32, kind="ExternalOutput")
    # DRAM scratch for partition-broadcasting softmax denominators
    # (SBUF->SBUF DMA cannot broadcast across partitions; DRAM can).
    rsum_dram = nc.dram_tensor("rsum_scratch", [8, 1024], f32, kind="Internal")
    rrec_dram = nc.dram_tensor("rrec_scratch", [8, 1024], f32, kind="Internal")

    KT = D // P  # 8 contraction tiles for projections
    ST = S // P  # 16 seq tiles
    VW = HPC * (DK + 1)  # 260: V columns + ones column per head

    with tile.TileContext(nc) as tc:
        with ExitStack() as ctx:
            # ---- pools (whole-kernel lifetime) ----
            # x pool holds xk/xq as [128, 1024] column chunks; the mask
            # tiles rotate into the xk buffers (dead after the lead) and
            # the maskneg tiles into the xq buffers (xq-c0 dead after
            # lead; xq-c1 readers are C's early-kt Q injections, emitted
            # before the first maskneg tile that reuses those buffers).
            x_pool = ctx.enter_context(tc.tile_pool(name="x", bufs=32))
            xv_pool = ctx.enter_context(tc.tile_pool(name="xv", bufs=8))
            w_pool = ctx.enter_context(tc.tile_pool(name="w", bufs=24))
            wo_pool = ctx.enter_context(tc.tile_pool(name="wo", bufs=2))
            id_pool = ctx.enter_context(tc.tile_pool(name="id", bufs=1))
            qt_pool = ctx.enter_context(tc.tile_pool(name="qt", bufs=2))
            kt_pool = ctx.enter_context(tc.tile_pool(name="kt", bufs=2))
            v_pool = ctx.enter_context(tc.tile_pool(name="v", bufs=ST))
            out_pool = ctx.enter_context(tc.tile_pool(name="outsb", bufs=2))
            exp_pool = ctx.enter_context(tc.tile_pool(name="exp", bufs=6))
            p_pool = ctx.enter_context(tc.tile_pool(name="p", bufs=8))
            otmp_pool = ctx.enter_context(tc.tile_pool(name="otmp", bufs=2))
            bc_pool = ctx.enter_context(tc.tile_pool(name="bc", bufs=2))
            r_pool = ctx.enter_context(tc.tile_pool(name="r", bufs=4))
            yev_pool = ctx.enter_context(tc.tile_pool(name="yev", bufs=4))
            dmy_pool = ctx.enter_context(tc.tile_pool(name="dmy", bufs=1))
            ps_s = ctx.enter_context(tc.tile_pool(name="ps_s", bufs=2, space="PSUM"))
            ps_o = ctx.enter_context(tc.tile_pool(name="ps_o", bufs=2, space="PSUM"))

            # ---- dummy exp: pull the ACT table load off the critical path
            dmy = dmy_pool.tile([P, 8], bf16, tag="dmy", name="dmy")
            nc.vector.memset(dmy[:], 0.0)
            nc.scalar.activation(dmy[:], dmy[:], Exp)

            # ---- DMAs across THREE queues. The HWDGE rings (sync,
            # scalar) run ~130GB/s each; the GPSIMD SWDGE path measured
            # ~2.5x that, so xv + all remaining weights ride on it. The
            # scalar queue carries only the pre-first-exp critical set
            # (exps are in-order behind any scalar DMA).
            w_sb = {}

            def load_w(wname, t, q):
                tiles = []
                for k in range(KT):
                    wt = w_pool.tile([P, EPC], bf16, tag="w", name="w")
                    q.dma_start(wt[:], t[k * P:(k + 1) * P, :])
                    tiles.append(wt)
                w_sb[wname] = tiles

            xk_sb = [[None] * KT for _ in range(2)]
            xq_sb = [[None] * KT for _ in range(2)]

            def load_x(dst, src, c, k, q):
                t = x_pool.tile([P, 1024], bf16, tag="x", name="x")
                q.dma_start(t[:], src[k * P:(k + 1) * P,
                                      c * 1024:(c + 1) * 1024])
                dst[c][k] = t

            load_w("wk", wk, nc.sync)
            load_w("wq", wq, nc.scalar)
            for c in range(2):
                for k in range(KT):
                    load_x(xk_sb, xk, c, k,
                           nc.sync if k % 2 == 0 else nc.scalar)
            for k in range(KT):
                load_x(xq_sb, xq, 0, k,
                       nc.sync if k % 2 == 0 else nc.scalar)
            for k in range(KT):
                load_x(xq_sb, xq, 1, k,
                       nc.sync if k % 2 == 0 else nc.scalar)
            # gpsimd (SWDGE) ring: wv, xv (full tiles), wo, ident
            load_w("wv", wv, nc.gpsimd)
            xv_sb = []
            for k in range(KT):
                t = xv_pool.tile([P, S], bf16, tag="xv", name="xv")
                nc.gpsimd.dma_start(t[:], xv[k * P:(k + 1) * P, :])
                xv_sb.append(t)
            wo_sb = []
            for k in range(2):
                wt = wo_pool.tile([P, D], bf16, tag="wo", name="wo")
                nc.gpsimd.dma_start(wt[:], wo[k * P:(k + 1) * P, :])
                wo_sb.append(wt)
            id_sb = id_pool.tile([P, P], bf16, tag="id", name="id")
            nc.gpsimd.dma_start(id_sb[:], ident[:, :])

            # ---- projection helpers ----
            qt_sb = [qt_pool.tile([P, S], bf16, tag="qt", name="qt") for _ in range(2)]
            kt_sb = [kt_pool.tile([P, S], bf16, tag="kt", name="kt") for _ in range(2)]

            def emit_qk_proj(dst_tiles, wname, x_sb, q4, m):
                """One (m, q4) psum group for the Q or K projection."""
                c, half = q4 // 2, (q4 % 2) * 512
                ps = ps_s.tile([P, 1024], f32, tag="ps_s", name="ps_s")
                for k in range(KT):
                    nc.tensor.matmul(
                        ps[:, 0:512],
                        lhsT=w_sb[wname][k][:, m * P:(m + 1) * P],
                        rhs=x_sb[c][k][:, half:half + 512],
                        start=(k == 0),
                        stop=(k == KT - 1),
                    )
                nc.vector.tensor_copy(
                    dst_tiles[m][:, q4 * 512:(q4 + 1) * 512], ps[:, 0:512])

            v_sb = [None] * ST

            def emit_v_proj(m):
                """V tile m matmuls; returns the eviction closure, which
                the caller schedules between the NEXT step's two exps so
                the ACT-queue copy never waits on these matmuls."""
                vt = v_pool.tile([P, VW], bf16, tag="v", name="v")
                ps = ps_s.tile([P, 1024], f32, tag="ps_s", name="ps_s")
                for k in range(KT):
                    nc.tensor.matmul(
                        ps[:, 0:EPC],
                        lhsT=xv_sb[k][:, m * P:(m + 1) * P],
                        rhs=w_sb["wv"][k][:, :],
                        start=(k == 0),
                        stop=(k == KT - 1),
                    )
                vs = vt[:].rearrange("p (h x) -> p h x", h=HPC)
                v_sb[m] = vt

                def evict():
                    nc.scalar.activation(
                        vs[:, :, 0:DK],
                        ps[:, 0:EPC].rearrange("p (h x) -> p h x", h=HPC),
                        Copy,
                    )
                    nc.vector.memset(vs[:, :, DK:DK + 1], 1.0)
                return evict

            # ---- lead-in: K proj (all), Q proj q-half 0 ----
            for q4 in range(4):
                for m in range(2):
                    emit_qk_proj(kt_sb, "wk", xk_sb, q4, m)
            for q4 in range(2):
                for m in range(2):
                    emit_qk_proj(qt_sb, "wq", xq_sb, q4, m)

            # masks for qh=0 (sync queue), rotating into dead xk buffers
            mask_sb = []
            for kt in range(ST):
                mt = x_pool.tile([P, 1024], bf16, tag="x", name="mask")
                nc.sync.dma_start(mt[:], mk[kt * P:(kt + 1) * P, :])
                mask_sb.append(mt)

            # ---- output projection ----
            out_sb = [out_pool.tile([P, S], bf16, tag="outsb", name="outsb")
                      for _ in range(2)]

            def emit_oproj_chunk(m, qc, q0, evict_act, pool=None):
                sl = slice(q0 + qc * 512, q0 + (qc + 1) * 512)
                if pool is None:
                    pool = ps_s if qc == 0 else ps_o
                ps = pool.tile([P, 512], f32,
                               tag="ps_s" if pool is ps_s else "ps_o",
                               name="ps_op")
                for k in range(2):
                    nc.tensor.matmul(
                        ps[:, 0:512],
                        lhsT=wo_sb[k][:, m * P:(m + 1) * P],
                        rhs=out_sb[k][:, sl],
                        start=(k == 0),
                        stop=(k == 1),
                    )
                ev = yev_pool.tile([P, 512], f32, tag="yev", name="yev")
                if evict_act:
                    nc.scalar.activation(ev[:], ps[:, 0:512], Copy)
                    nc.sync.dma_start(y[m * P:(m + 1) * P, sl], ev[:])
                else:
                    nc.vector.tensor_copy(ev[:], ps[:, 0:512])
                    nc.scalar.dma_start(y[m * P:(m + 1) * P, sl], ev[:])

            # ---- attention ----
            maskneg_sb = [None] * ST

            def attention(qh, hp, inject):
                """One (q-half, head-pair-group) pass. inject[kt] is an
                optional callable emitted after that step's PV; if it
                returns a closure, that closure is emitted between the
                NEXT step's two exps (deferred ACT-queue eviction)."""
                po = [ps_o.tile([DK + 1, 1024], f32, tag="ps_o", name="ps_o")
                      for _ in range(2)]

                def emit_pv(kt, pts):
                    for h2 in range(2):
                        h = 2 * hp + h2
                        for qc in range(2):
                            nc.tensor.matmul(
                                po[h2][:, qc * 512:(qc + 1) * 512],
                                lhsT=v_sb[kt][:, h * (DK + 1):(h + 1) * (DK + 1)],
                                rhs=pts[h2][:, qc * 512:(qc + 1) * 512],
                                start=(kt == 0),
                                stop=(kt == 15),
                            )

                q0 = qh * 1024
                prev = None
                pending = None
                for kt in range(ST):
                    psts = [ps_s.tile([P, 1024], f32, tag="ps_s", name="ps_s")
                            for _ in range(2)]
                    if qh == 1:
                        # additive mask: identity-stationary matmuls seed
                        # the psum with -1e9 on masked elements
                        for h2 in range(2):
                            for qc in range(2):
                                nc.tensor.matmul(
                                    psts[h2][:, qc * 512:(qc + 1) * 512],
                                    lhsT=id_sb[:],
                                    rhs=maskneg_sb[kt][:, qc * 512:(qc + 1) * 512],
                                    start=True,
                                    stop=False,
                                )
                    # scores: qc-outer / h2-inner so the two half-array
                    # matmuls (rows 0-63 vs 64-127) run concurrently
                    for qc in range(2):
                        for h2 in range(2):
                            r0 = h2 * DK
                            nc.tensor.matmul(
                                psts[h2][:, qc * 512:(qc + 1) * 512],
                                lhsT=kt_sb[hp][r0:r0 + DK, kt * P:(kt + 1) * P],
                                rhs=qt_sb[hp][r0:r0 + DK,
                                              q0 + qc * 512:q0 + (qc + 1) * 512],
                                start=(qh == 0),
                                stop=True,
                                tile_position=(r0, 0),
                            )
                    pts = []
                    for h2 in range(2):
                        et = exp_pool.tile([P, 1024], bf16, tag="exp", name="exp")
                        nc.scalar.activation(et[:], psts[h2][:], Exp, scale=0.125)
                        if h2 == 0 and pending is not None:
                            pending()
                            pending = None
                        if qh == 0:
                            pt = p_pool.tile([P, 1024], bf16, tag="p", name="p")
                            nc.vector.tensor_mul(pt[:], et[:], mask_sb[kt][:])
                            pts.append(pt)
                        else:
                            pts.append(et)
                    # prefetch next-half (additive) mask during (qh0, hp1)
                    if qh == 0 and hp == 1:
                        mt = x_pool.tile([P, 1024], bf16, tag="x", name="maskn")
                        nc.sync.dma_start(mt[:], mn[kt * P:(kt + 1) * P, :])
                        maskneg_sb[kt] = mt
                    if prev is not None:
                        emit_pv(kt - 1, prev)
                    if inject[kt] is not None:
                        pending = inject[kt]()
                    prev = pts
                if pending is not None:
                    pending()
                emit_pv(15, prev)

                # normalize: row DK of po is sum_k p[k, q]. The chained
                # DMAs run on the GPSIMD (SWDGE) queue: their round-trip
                # semaphore waits would head-of-line block every later
                # transfer on a HWDGE queue. The two h2 chains are
                # interleaved stage-by-stage so the round trips overlap.
                ots, rqs, bcs, rrs = [], [], [], []
                for h2 in range(2):
                    ot = otmp_pool.tile([DK + 1, 1024], f32, tag="otmp",
                                        name="otmp")
                    nc.vector.tensor_copy(ot[:], po[h2][:])
                    ots.append(ot)
                for h2 in range(2):
                    ridx = (qh * 2 + hp) * 2 + h2
                    nc.gpsimd.dma_start(
                        rsum_dram[ridx:ridx + 1, :], ots[h2][DK:DK + 1, :])
                for h2 in range(2):
                    ridx = (qh * 2 + hp) * 2 + h2
                    rr = r_pool.tile([P, 8], f32, tag="r", name="rr")
                    nc.gpsimd.dma_start(
                        rr[:],
                        rsum_dram[ridx:ridx + 1, :].rearrange(
                            "o (p f) -> (o p) f", p=P),
                    )
                    rrs.append(rr)
                for h2 in range(2):
                    rq = r_pool.tile([P, 8], f32, tag="r", name="rq")
                    nc.vector.reciprocal(rq[:], rrs[h2][:])
                    rqs.append(rq)
                for h2 in range(2):
                    ridx = (qh * 2 + hp) * 2 + h2
                    nc.gpsimd.dma_start(
                        rrec_dram[ridx:ridx + 1, :].rearrange(
                            "o (p f) -> (o p) f", p=P),
                        rqs[h2][:],
                    )
                for h2 in range(2):
                    ridx = (qh * 2 + hp) * 2 + h2
                    bc = bc_pool.tile([DK, 1024], f32, tag="bc", name="bc")
                    nc.gpsimd.dma_start(
                        bc[:],
                        rrec_dram[ridx:ridx + 1, :].broadcast_to([DK, 1024]),
                    )
                    bcs.append(bc)
                for h2 in range(2):
                    nc.gpsimd.tensor_mul(
                        out_sb[hp][h2 * DK:(h2 + 1) * DK, q0:q0 + 1024],
                        ots[h2][0:DK, :],
                        bcs[h2][:],
                    )

            # phase B: (qh=0, hp=0), V projection injected 1 tile per kt
            attention(0, 0, [lambda m=m: emit_v_proj(m) for m in range(ST)])
            # phase C: (qh=0, hp=1), Q projection q-half 1 injected on
            # EARLY kts (before maskneg prefetches recycle xq-c1 buffers)
            c_inject = [None] * ST
            for i, (q4, m) in enumerate(((2, 0), (2, 1), (3, 0), (3, 1))):
                c_inject[1 + 2 * i] = (
                    lambda q4=q4, m=m: emit_qk_proj(qt_sb, "wq", xq_sb, q4, m))
            attention(0, 1, c_inject)
            # phases E, F: (qh=1, hp=0/1), additive mask
            attention(1, 0, [None] * ST)
            attention(1, 1, [None] * ST)
            # tail: all 64 output-projection chunks, dense. q-half 0
            # first (its normalization finished two phases ago) so the
            # PE streams while F's normalization chain completes.
            i = 0
            for q0 in (0, 1024):
                for m in range(D // P):
                    for qc in range(2):
                        emit_oproj_chunk(m, qc, q0,
                                         evict_act=(i % 2 == 0))
                        i += 1

    _split_multi_waits(nc)
    return nc


def _get_nc():
    if "nc" not in _CACHE:
        _CACHE["nc"] = _build_bass()
    return _CACHE["nc"]


def kernel(query, key, value, mask, w_q, w_k, w_v, w_o, **unused):
    nc = _get_nc()
    from concourse.bass_utils import run_bass_kernel_spmd

    ident = np.eye(P, dtype=BF16)
    in_maps = []
    for c in range(N_CORES):
        b = c // (N_CORES // B)
        hg = c % (N_CORES // B)
        e0 = hg * EPC
        mt = np.ascontiguousarray(mask[b].T).astype(np.float32)
        in_maps.append({
            "xq_t": np.ascontiguousarray(query[b].T).astype(BF16),
            "xk_t": np.ascontiguousarray(key[b].T).astype(BF16),
            "xv_t": np.ascontiguousarray(value[b].T).astype(BF16),
            "mask_t": np.ascontiguousarray(mt[:, 0:1024]).astype(BF16),
            "maskneg_t": np.ascontiguousarray(
                (mt[:, 1024:2048] - 1.0) * 1e9).astype(BF16),
            "ident_t": ident,
            "wq_t": np.ascontiguousarray(w_q[e0:e0 + EPC, :].T).astype(BF16),
            "wk_t": np.ascontiguousarray(w_k[e0:e0 + EPC, :].T).astype(BF16),
            "wv_t": np.ascontiguousarray(w_v[e0:e0 + EPC, :].T).astype(BF16),
            "wo_t": np.ascontiguousarray(w_o[:, e0:e0 + EPC].T).astype(BF16),
        })

    res = run_bass_kernel_spmd(nc, in_maps, core_ids=list(range(N_CORES)))
    _CACHE["last_results"] = res

    gpb = N_CORES // B
    out = np.empty((B, S, D), dtype=np.float32)
    for b in range(B):
        acc = res.results[b * gpb]["y_t"].astype(np.float32)
        for c in range(b * gpb + 1, (b + 1) * gpb):
            acc = acc + res.results[c]["y_t"]
        out[b] = acc.T
    return out
